# revision 38
# baseline (speedup 1.0000x reference)
"""Trainium2 Bass kernel for nn_AstraloraLayer: y = x @ A.T, A = w.reshape(512, 512).

Sharding: data-parallel over the flattened token dim. x (8, 8192, 512) -> 65536
tokens, 8192 per core; w replicated (U,S,V unused in the forward). The host
pre-transposes each x shard to [512, 8192] so the contraction dim (d_in) lands
on SBUF partitions with fully contiguous DMA, and feeds A.T [d_in, d_out] so
weight chunks load naturally. Inputs/outputs travel as bf16 (f32 PSUM
accumulation; rel err ~3e-3 vs the f32 reference), halving HBM traffic and
doubling PE rate vs fp32. Output returns in natural [tokens, d_out] layout.

Per core: 64 token tiles of 128; each tile is a 4-matmul K-accumulation
(512 = 4 x 128, x tile stationary / A.T chunk moving) into one of 4 rotating
PSUM banks. Engine programs:
  SP  - cold-path primer DMAs, weights, then x units in consumption order
        (tapered: small head units so the PE starts early)
  ACT - primers, then batched output DMAs (second HWDGE ring), tapered at
        the tail
  PE  - weight fence first, then dense back-to-back matmul groups
  DVE - PSUM -> SBUF bf16 casts into rotating output slots
  POOL- final semaphore clears (leave a clean state for re-execution)

First-execution hardening (measured ~50% corruption rate without): the first
few DMAs after NEFF load can signal completion before their data is readable,
so (a) N_PRIME dummy DMAs absorb the cold path on each ring, (b) the PE
consumes x unit u only after unit u+X_LAG signals, (c) all semaphores are
cleared at kernel end so re-executions never see stale-hot counts.
"""

import numpy as np

import concourse.bass as bass
import concourse.mybir as mybir
from concourse.bass_utils import run_bass_kernel_spmd

N_CORES = 8
D_IN = 512
D_OUT = 512
TOK = 8192  # tokens per core
KC = 128  # contraction chunk (partition dim)
NK = D_IN // KC  # 4
TT = TOK // 128  # total matmul tiles (64)
NPS = 4  # rotating PSUM banks
OBT = 2  # tiles per output DMA
NOB = 8  # output staging slots
X_LAG = 3  # consume x unit u only once unit u+X_LAG has signalled
N_PRIME = 2  # cold-path primer DMAs per HWDGE ring before any real transfer
N_WARM = 7  # HAM-prewarm dummy matmuls between the weight fence and x wait

# x DMA units in tokens, all on the SP ring in consumption order; small head
# units keep the pipeline fill short even with the X_LAG safety margin.
X_UNITS = [128] * 4 + [256] * 4 + [512] * 13
assert sum(X_UNITS) == TOK

COMPUTE = "bf16"

def build_kernel(compute=COMPUTE):
    if compute == "bf16":
        in_dt = mybir.dt.bfloat16
        out_dt = mybir.dt.bfloat16
    elif compute == "f32r":
        in_dt = mybir.dt.float32r
        out_dt = mybir.dt.float32
    else:
        in_dt = mybir.dt.float32
        out_dt = mybir.dt.float32

    nc = bass.Bass()
    xT = nc.declare_dram_parameter("xT", [D_IN, TOK], in_dt, isOutput=False)
    aT = nc.declare_dram_parameter("aT", [D_IN, D_OUT], in_dt, isOutput=False)
    out = nc.declare_dram_parameter("out", [TOK, D_OUT], out_dt, isOutput=True)

    n_xu = len(X_UNITS)
    # tile g -> x unit threshold (with X_LAG safety margin)
    x_thresh = [0] * TT
    tok0 = 0
    for u, n in enumerate(X_UNITS):
        for t in range(tok0 // 128, (tok0 + n) // 128):
            x_thresh[t] = 16 * (min(u + X_LAG, n_xu - 1) + 1)
        tok0 += n

    # output DMA units in tiles, tapered at the end to cut the final drain
    out_units = [OBT] * (TT // OBT - 1) + [1] * OBT
    n_out_dma = len(out_units)

    with (
        nc.sbuf_tensor([KC, NK * D_OUT], in_dt) as wsb,
        nc.sbuf_tensor([KC, NK * TOK], in_dt) as xsb,
        nc.sbuf_tensor([128, NOB * OBT * D_OUT], out_dt) as obuf,
        nc.sbuf_tensor([128, 2 * N_PRIME * 64], in_dt) as prime,
        nc.sbuf_tensor([128, D_OUT + 128], in_dt) as warm,
        nc.psum_tensor([128, D_OUT], mybir.dt.float32) as ps0,
        nc.psum_tensor([128, D_OUT], mybir.dt.float32) as ps1,
        nc.psum_tensor([128, D_OUT], mybir.dt.float32) as ps2,
        nc.psum_tensor([128, D_OUT], mybir.dt.float32) as ps3,
        nc.psum_tensor([128, D_OUT], mybir.dt.float32) as ps_warm,
        nc.semaphore("prime_sem") as prime_sem,
        nc.semaphore("w_sem") as w_sem,
        nc.semaphore("xs_sem") as xs_sem,
        nc.semaphore("mm_sem") as mm_sem,
        nc.semaphore("cp_sem") as cp_sem,
        nc.semaphore("o_sem") as o_sem,
        nc.Block(no_gpsimd_drain=True) as block,
    ):
        ps = [ps0, ps1, ps2, ps3]

        @block.sync
        def _(sync):
            # cold-path primers: the first DMAs after NEFF load can signal
            # completion before their data is visible; burn that on dummies
            for i in range(N_PRIME):
                sync.dma_start(
                    out=prime[:, i * 64 : (i + 1) * 64],
                    in_=xT[:KC, i * 64 : (i + 1) * 64],
                ).then_inc(prime_sem, 16)
            # first half of the weights; the other half lands in parallel
            # on the ACT ring (they gate the PE fence)
            sync.dma_start(
                out=wsb[:, : 2 * D_OUT].rearrange("p (k o) -> p k o", k=2),
                in_=aT[: 2 * KC, :].rearrange("(k p) o -> p k o", p=KC),
            ).then_inc(w_sem, 16)
            tok0 = 0
            for n in X_UNITS:
                sync.dma_start(
                    out=xsb[:, :]
                    .rearrange("p (k t) -> p k t", k=NK)[:, :, tok0 : tok0 + n],
                    in_=xT[:, tok0 : tok0 + n].rearrange("(k p) t -> p k t", p=KC),
                ).then_inc(xs_sem, 16)
                tok0 += n

        @block.tensor
        def _(tensor):
            # weight fence as the PE's first instruction: nothing (LDWEIGHTS
            # included) can be scheduled above it
            tensor.wait_ge(w_sem, 32)
            # HAM prewarm while the first x units land: garbage matmuls into a
            # scratch bank lift the PE clock gate before the real stream
            for _ in range(N_WARM):
                tensor.matmul(
                    ps_warm[:, :],
                    warm[:, D_OUT : D_OUT + 128],
                    warm[:, 0:D_OUT],
                    start=True,
                    stop=True,
                )
            for g in range(TT):
                tensor.wait_ge(xs_sem, x_thresh[g])
                if g >= NPS:
                    tensor.wait_ge(cp_sem, g - NPS + 1)
                for k in range(NK):
                    mm = tensor.matmul(
                        ps[g % NPS][:, :],
                        xsb[:, k * TOK + g * 128 : k * TOK + (g + 1) * 128],
                        wsb[:, k * D_OUT : (k + 1) * D_OUT],
                        start=(k == 0),
                        stop=(k == NK - 1),
                    )
                mm.then_inc(mm_sem, 1)

        @block.vector
        def _(vector):
            for g in range(TT):
                j = g // OBT
                slot = j % NOB
                pos = g % OBT
                vector.wait_ge(mm_sem, g + 1)
                if pos == 0 and j >= NOB:
                    vector.wait_ge(o_sem, 16 * (j - NOB + 1))
                vector.tensor_copy(
                    out=obuf[
                        :,
                        (slot * OBT + pos) * D_OUT : (slot * OBT + pos + 1) * D_OUT,
                    ],
                    in_=ps[g % NPS][:, :],
                ).then_inc(cp_sem, 1)

        @block.scalar
        def _(scalar):
            for i in range(N_PRIME):
                scalar.dma_start(
                    out=prime[:, (N_PRIME + i) * 64 : (N_PRIME + i + 1) * 64],
                    in_=xT[:KC, i * 64 : (i + 1) * 64],
                ).then_inc(prime_sem, 16)
            # second half of the weights on this ring, parallel with SP's half
            scalar.dma_start(
                out=wsb[:, 2 * D_OUT :].rearrange("p (k o) -> p k o", k=2),
                in_=aT[2 * KC :, :].rearrange("(k p) o -> p k o", p=KC),
            ).then_inc(w_sem, 16)
            g0 = 0
            for sz in out_units:
                scalar.wait_ge(cp_sem, g0 + sz)
                tok0 = g0 * 128
                col0 = ((g0 // OBT) % NOB) * OBT + (g0 % OBT)
                scalar.dma_start(
                    out=out[tok0 : tok0 + sz * 128, :].rearrange(
                        "(a p) o -> p a o", p=128
                    ),
                    in_=obuf[:, col0 * D_OUT : (col0 + sz) * D_OUT].rearrange(
                        "p (a o) -> p a o", a=sz
                    ),
                ).then_inc(o_sem, 16)
                g0 += sz
            # completion wait lives on gpsimd (before its sem clears); a
            # trailing wait here would race those clears

        @block.gpsimd
        def _(gpsimd):
            # Leave every kernel semaphore at 0 for the next execution so a
            # re-run can never see stale-hot counts (trivially-satisfied waits
            # reading not-yet-DMA'd SBUF).
            gpsimd.wait_ge(o_sem, 16 * n_out_dma)
            for sem in (prime_sem, w_sem, xs_sem, mm_sem, cp_sem, o_sem):
                gpsimd.sem_clear(sem)

    return nc


def _prep_inputs(x, w, compute=COMPUTE):
    if compute == "bf16":
        import ml_dtypes

        np_dt = ml_dtypes.bfloat16
    else:
        np_dt = np.float32
    xf = np.asarray(x, dtype=np.float32).reshape(-1, D_IN)
    A = np.asarray(w, dtype=np.float32).reshape(D_OUT, D_IN)
    aT = np.ascontiguousarray(A.T).astype(np_dt)
    in_maps = []
    for s in range(N_CORES):
        xs = xf[s * TOK : (s + 1) * TOK]
        in_maps.append({"xT": np.ascontiguousarray(xs.T).astype(np_dt), "aT": aT})
    return in_maps


def _gather_output(results, like_shape):
    y = np.concatenate(
        [np.asarray(results[i]["out"], dtype=np.float32) for i in range(N_CORES)],
        axis=0,
    )
    return y.reshape(*like_shape[:-1], D_OUT)


def kernel(x, w, U=None, S=None, V=None, **_):
    nc = build_kernel()
    in_maps = _prep_inputs(x, w)
    res = run_bass_kernel_spmd(nc, in_maps, core_ids=list(range(N_CORES)))
    return _gather_output(res.results, x.shape)


# revision 39
# speedup vs baseline: 1.1917x; 1.1917x over previous
"""Trainium2 Bass kernel for nn_AstraloraLayer: y = x @ A.T, A = w.reshape(512, 512).

Sharding: data-parallel over the flattened token dim. x (8, 8192, 512) -> 65536
tokens, 8192 per core; w replicated (U,S,V unused in the forward). The host
pre-transposes each x shard to [512, 8192] so the contraction dim (d_in) lands
on SBUF partitions with fully contiguous DMA, and feeds A.T [d_in, d_out] so
weight chunks load naturally. Inputs/outputs travel as bf16 (f32 PSUM
accumulation; rel err ~3e-3 vs the f32 reference), halving HBM traffic and
doubling PE rate vs fp32. Output returns in natural [tokens, d_out] layout.

Per core: 64 token tiles of 128; each tile is a 4-matmul K-accumulation
(512 = 4 x 128, x tile stationary / A.T chunk moving) into one of 4 rotating
PSUM banks. Engine programs:
  SP  - cold-path primer DMAs, weights, then x units in consumption order
        (tapered: small head units so the PE starts early)
  ACT - primers, then batched output DMAs (second HWDGE ring), tapered at
        the tail
  PE  - weight fence first, then dense back-to-back matmul groups
  DVE - PSUM -> SBUF bf16 casts into rotating output slots
  POOL- final semaphore clears (leave a clean state for re-execution)

First-execution hardening (measured ~50% corruption rate without): the first
few DMAs after NEFF load can signal completion before their data is readable,
so (a) N_PRIME dummy DMAs absorb the cold path on each ring, (b) the PE
consumes x unit u only after unit u+X_LAG signals, (c) all semaphores are
cleared at kernel end so re-executions never see stale-hot counts.
"""

import numpy as np

import concourse.bass as bass
import concourse.mybir as mybir
from concourse.bass_utils import run_bass_kernel_spmd

N_CORES = 8
D_IN = 512
D_OUT = 512
TOK = 8192  # tokens per core
KC = 128  # contraction chunk (partition dim)
NK = D_IN // KC  # 4
TT = TOK // 128  # total matmul tiles (64)
NPS = 4  # rotating PSUM banks
OBT = 2  # tiles per output DMA
NOB = 8  # output staging slots
X_LAG = 3  # consume x unit u only once unit u+X_LAG has signalled
N_PRIME = 2  # cold-path primer DMAs per HWDGE ring before any real transfer
N_WARM = 6  # HAM-prewarm dummy matmuls between the weight fence and x wait

# x DMA units in tokens, all on the SP ring in consumption order; small head
# units keep the pipeline fill short even with the X_LAG safety margin.
X_UNITS = [128] * 4 + [256] * 4 + [512] * 13
assert sum(X_UNITS) == TOK

COMPUTE = "bf16"

def build_kernel(compute=COMPUTE):
    if compute == "bf16":
        in_dt = mybir.dt.bfloat16
        out_dt = mybir.dt.bfloat16
    elif compute == "f32r":
        in_dt = mybir.dt.float32r
        out_dt = mybir.dt.float32
    else:
        in_dt = mybir.dt.float32
        out_dt = mybir.dt.float32

    nc = bass.Bass()
    xT = nc.declare_dram_parameter("xT", [D_IN, TOK], in_dt, isOutput=False)
    aT = nc.declare_dram_parameter("aT", [D_IN, D_OUT], in_dt, isOutput=False)
    out = nc.declare_dram_parameter("out", [TOK, D_OUT], out_dt, isOutput=True)

    n_xu = len(X_UNITS)
    # tile g -> x unit threshold (with X_LAG safety margin)
    x_thresh = [0] * TT
    tok0 = 0
    for u, n in enumerate(X_UNITS):
        for t in range(tok0 // 128, (tok0 + n) // 128):
            x_thresh[t] = 16 * (min(u + X_LAG, n_xu - 1) + 1)
        tok0 += n

    # output DMA units in tiles, tapered at the end to cut the final drain
    out_units = [OBT] * (TT // OBT - 1) + [1] * OBT
    n_out_dma = len(out_units)

    with (
        nc.sbuf_tensor([KC, NK * D_OUT], in_dt) as wsb,
        nc.sbuf_tensor([KC, NK * TOK], in_dt) as xsb,
        nc.sbuf_tensor([128, NOB * OBT * D_OUT], out_dt) as obuf,
        nc.sbuf_tensor([128, 2 * N_PRIME * 64], in_dt) as prime,
        nc.sbuf_tensor([128, D_OUT + 128], in_dt) as warm,
        nc.psum_tensor([128, D_OUT], mybir.dt.float32) as ps0,
        nc.psum_tensor([128, D_OUT], mybir.dt.float32) as ps1,
        nc.psum_tensor([128, D_OUT], mybir.dt.float32) as ps2,
        nc.psum_tensor([128, D_OUT], mybir.dt.float32) as ps3,
        nc.psum_tensor([128, D_OUT], mybir.dt.float32) as ps_warm,
        nc.semaphore("prime_sem") as prime_sem,
        nc.semaphore("w_sem") as w_sem,
        nc.semaphore("xs_sem") as xs_sem,
        nc.semaphore("mm_sem") as mm_sem,
        nc.semaphore("cp_sem") as cp_sem,
        nc.semaphore("o_sem") as o_sem,
        nc.Block(no_gpsimd_drain=True) as block,
    ):
        ps = [ps0, ps1, ps2, ps3]

        @block.sync
        def _(sync):
            # cold-path primers: the first DMAs after NEFF load can signal
            # completion before their data is visible; burn that on dummies
            for i in range(N_PRIME):
                sync.dma_start(
                    out=prime[:, i * 64 : (i + 1) * 64],
                    in_=xT[:KC, i * 64 : (i + 1) * 64],
                ).then_inc(prime_sem, 16)
            # first half of the weights; the other half lands in parallel
            # on the ACT ring (they gate the PE fence)
            sync.dma_start(
                out=wsb[:, : 2 * D_OUT].rearrange("p (k o) -> p k o", k=2),
                in_=aT[: 2 * KC, :].rearrange("(k p) o -> p k o", p=KC),
            ).then_inc(w_sem, 16)
            tok0 = 0
            for n in X_UNITS:
                sync.dma_start(
                    out=xsb[:, :]
                    .rearrange("p (k t) -> p k t", k=NK)[:, :, tok0 : tok0 + n],
                    in_=xT[:, tok0 : tok0 + n].rearrange("(k p) t -> p k t", p=KC),
                ).then_inc(xs_sem, 16)
                tok0 += n

        @block.tensor
        def _(tensor):
            # weight fence as the PE's first instruction: nothing (LDWEIGHTS
            # included) can be scheduled above it
            tensor.wait_ge(w_sem, 32)
            # HAM prewarm while the first x units land: garbage matmuls into a
            # scratch bank lift the PE clock gate before the real stream
            for _ in range(N_WARM):
                tensor.matmul(
                    ps_warm[:, :],
                    warm[:, D_OUT : D_OUT + 128],
                    warm[:, 0:D_OUT],
                    start=True,
                    stop=True,
                )
            last_thresh = 0
            for g in range(TT):
                # waits on an already-seen threshold are redundant (monotone
                # semaphore) and just cost NX issue time between matmuls
                if x_thresh[g] > last_thresh:
                    tensor.wait_ge(xs_sem, x_thresh[g])
                    last_thresh = x_thresh[g]
                if g >= NPS:
                    tensor.wait_ge(cp_sem, g - NPS + 1)
                for k in range(NK):
                    mm = tensor.matmul(
                        ps[g % NPS][:, :],
                        xsb[:, k * TOK + g * 128 : k * TOK + (g + 1) * 128],
                        wsb[:, k * D_OUT : (k + 1) * D_OUT],
                        start=(k == 0),
                        stop=(k == NK - 1),
                    )
                mm.then_inc(mm_sem, 1)

        @block.vector
        def _(vector):
            for g in range(TT):
                j = g // OBT
                slot = j % NOB
                pos = g % OBT
                vector.wait_ge(mm_sem, g + 1)
                if pos == 0 and j >= NOB:
                    vector.wait_ge(o_sem, 16 * (j - NOB + 1))
                vector.tensor_copy(
                    out=obuf[
                        :,
                        (slot * OBT + pos) * D_OUT : (slot * OBT + pos + 1) * D_OUT,
                    ],
                    in_=ps[g % NPS][:, :],
                ).then_inc(cp_sem, 1)

        @block.scalar
        def _(scalar):
            for i in range(N_PRIME):
                scalar.dma_start(
                    out=prime[:, (N_PRIME + i) * 64 : (N_PRIME + i + 1) * 64],
                    in_=xT[:KC, i * 64 : (i + 1) * 64],
                ).then_inc(prime_sem, 16)
            # second half of the weights on this ring, parallel with SP's half
            scalar.dma_start(
                out=wsb[:, 2 * D_OUT :].rearrange("p (k o) -> p k o", k=2),
                in_=aT[2 * KC :, :].rearrange("(k p) o -> p k o", p=KC),
            ).then_inc(w_sem, 16)
            g0 = 0
            for sz in out_units:
                scalar.wait_ge(cp_sem, g0 + sz)
                tok0 = g0 * 128
                col0 = ((g0 // OBT) % NOB) * OBT + (g0 % OBT)
                scalar.dma_start(
                    out=out[tok0 : tok0 + sz * 128, :].rearrange(
                        "(a p) o -> p a o", p=128
                    ),
                    in_=obuf[:, col0 * D_OUT : (col0 + sz) * D_OUT].rearrange(
                        "p (a o) -> p a o", a=sz
                    ),
                ).then_inc(o_sem, 16)
                g0 += sz
            # completion wait lives on gpsimd (before its sem clears); a
            # trailing wait here would race those clears

        @block.gpsimd
        def _(gpsimd):
            # Leave every kernel semaphore at 0 for the next execution so a
            # re-run can never see stale-hot counts (trivially-satisfied waits
            # reading not-yet-DMA'd SBUF).
            gpsimd.wait_ge(o_sem, 16 * n_out_dma)
            for sem in (prime_sem, w_sem, xs_sem, mm_sem, cp_sem, o_sem):
                gpsimd.sem_clear(sem)

    return nc


def _prep_inputs(x, w, compute=COMPUTE):
    if compute == "bf16":
        import ml_dtypes

        np_dt = ml_dtypes.bfloat16
    else:
        np_dt = np.float32
    xf = np.asarray(x, dtype=np.float32).reshape(-1, D_IN)
    A = np.asarray(w, dtype=np.float32).reshape(D_OUT, D_IN)
    aT = np.ascontiguousarray(A.T).astype(np_dt)
    in_maps = []
    for s in range(N_CORES):
        xs = xf[s * TOK : (s + 1) * TOK]
        in_maps.append({"xT": np.ascontiguousarray(xs.T).astype(np_dt), "aT": aT})
    return in_maps


def _gather_output(results, like_shape):
    y = np.concatenate(
        [np.asarray(results[i]["out"], dtype=np.float32) for i in range(N_CORES)],
        axis=0,
    )
    return y.reshape(*like_shape[:-1], D_OUT)


def kernel(x, w, U=None, S=None, V=None, **_):
    nc = build_kernel()
    in_maps = _prep_inputs(x, w)
    res = run_bass_kernel_spmd(nc, in_maps, core_ids=list(range(N_CORES)))
    return _gather_output(res.results, x.shape)
